# revision 31
# baseline (speedup 1.0000x reference)
"""AsyncCrossModalConsistencyLoss distributed Bass kernel for 8 TRN2 NeuronCores.

Data-parallel: batch dim (B=8) sharded one element per core. Each core:
  - streams its [4096, 512] visual/audio shard via SWDGE (nc.gpsimd)
    1 MB chunks casting f32->bf16 in flight (lightest compute load; the
    HWDGE-f32 path with on-chip casts measured slower end-to-end because
    the extra 8 cast passes/chunk overload DVE), tail-tapered 2/1/1
    tiles to shorten the post-DMA critical path
  - per [128,512] tile: row sum-of-squares split 4/4 across engines
    (all v-squares on ScalarE Square accum, all a-squares on DVE
    scalar_tensor_tensor accum -- swept on HW: ScalarE activation+accum
    costs ~2x a DVE stt pass, and InstTensorTensorReduce / InstBNStats
    fault on this HW),
    prod=v*a (DVE bf16, paired 2-tile ops), batched 1/max(norm,eps)
    with the reciprocal written bf16 directly (weights feed bf16
    matmuls; drops an f32 intermediate + copy per chunk),
    then TensorE matmuls accumulate sum_s v_hat / sum_s a_hat and the
    sync dot-sum in PSUM; suma's chain stops first in the final tile so
    the epilogue's PSUM->SBUF copy overlaps the remaining matmuls
  - 5-op epilogue: suma copy (ScalarE), total=<sumv,suma> (stt accum),
    sync=reduce(sync_ps), z=cA*total+cS*sync (stt against per-core
    host-precomputed constants), loss/8 = max(z+cM, 0) on DVE (avoids
    an ACT table switch); an initial dummy Sqrt pins the one ACT table
    set (sqrt_and_others) that covers both Square and Sqrt
AllReduce(add) over the 8 cores produces the global mean loss.

Host precompute (per core, from target w in {0,1}): sgn = 2w-1,
  cA = sgn*C_ASYNC/8, cS = -sgn*(C_SYNC+C_ASYNC)/8,
  cM = (0.1 + 0.9*w)*MARGIN/8
so that loss/8 = relu(cA*total + cS*sync + cM) exactly matches
  w*relu(async-sync+M) + (1-w)*relu(sync-async+0.1M), scaled by 1/8.
"""

import numpy as np

import concourse.bass as bass
import concourse.tile as tile
from concourse import bacc, mybir
from concourse.bass_utils import run_bass_kernel_spmd

N_CORES = 8
S = 4096
D = 512
P = 128
NT = S // P              # 32 compute tiles of [128, 512]
FREE = NT * D            # 16384 columns per partition

# tiles per DMA/compute chunk; 1 MB bulk chunks, tapered tail
PLAN = (4, 4, 4, 4, 4, 4, 4, 2, 1, 1)
assert sum(PLAN) == NT

EPS_DIV = 1e-8
MARGIN = 0.5
C_SYNC = 1.0 / S
C_ASYNC = 1.0 / (S * (S - 1) + EPS_DIV)

F32 = mybir.dt.float32
BF16 = mybir.dt.bfloat16
AF = mybir.ActivationFunctionType
OP = mybir.AluOpType


_ACT_ASQ = [0]
_ACT_VSQ = [4]
_PROD_W = [2]
_CPLAN = [None]
_RECIP_BF = [True]
_PLAN = [PLAN]
_SQ_MODE = ["act"]


def _build(collective=True, reps=1, dma_mode="swdge_cast", pool_casts=0,
           act_relu=False, act_asq=0, act_vsq=4, plan=None, sq_mode="act",
           prod_w=2, compute_plan=None, recip_bf=True):
    """reps>1: wrap the body in tc.For_i for differential timing (no
    collective in that mode — collectives can't sit in control flow)."""
    import contextlib

    _ACT_ASQ[0] = act_asq
    _ACT_VSQ[0] = act_vsq
    _PROD_W[0] = prod_w
    _CPLAN[0] = tuple(compute_plan) if compute_plan is not None else None
    _RECIP_BF[0] = recip_bf
    assert _CPLAN[0] is None or sum(_CPLAN[0]) == NT
    _PLAN[0] = tuple(plan) if plan is not None else PLAN
    _SQ_MODE[0] = sq_mode
    assert sum(_PLAN[0]) == NT

    nc = bacc.Bacc(
        "TRN2", target_bir_lowering=False, debug=False,
        num_devices=N_CORES if collective else 1,
    )
    v_ext = nc.dram_tensor("v", [S, D], F32, kind="ExternalInput")
    a_ext = nc.dram_tensor("a", [S, D], F32, kind="ExternalInput")
    c_ext = nc.dram_tensor("c", [1, 3], F32, kind="ExternalInput")
    out_ext = nc.dram_tensor("out", [1, 1], F32, kind="ExternalOutput")

    # Row s = p*NT + n lands on partition p, tile n: contiguous 64KB per
    # partition in DRAM -> ideal DMA pattern. Any row->(p,n) bijection works
    # because every reduction here is symmetric over rows.
    v_re = v_ext.ap().rearrange("(p n) d -> p (n d)", p=P)
    a_re = a_ext.ap().rearrange("(p n) d -> p (n d)", p=P)

    with tile.TileContext(nc) as tc:
        with (
            tc.tile_pool(name="big", bufs=1) as big,
            tc.tile_pool(name="scratch", bufs=2) as scratch,
            tc.tile_pool(name="small", bufs=3) as small,
            tc.tile_pool(name="psum", bufs=1, space="PSUM") as psum,
            tc.tile_pool(name="dram", bufs=1, space="DRAM") as dram,
        ):
            sb_dt = F32 if dma_mode == "hwdge_f32" else BF16
            v_sb = big.tile([P, FREE], sb_dt)
            a_sb = big.tile([P, FREE], sb_dt)
            c_sb = big.tile([1, 3], F32)
            eps_b = big.tile([P, 1], F32)
            nc.vector.memset(eps_b[:], 1e-24)
            nc.sync.dma_start(c_sb[:], c_ext[:])
            # Warm the ACT table with a set that has BOTH Sqrt and Square
            # (sqrt_and_others); otherwise bass loads a square-only set
            # first and pays a second ~2.7us load before the first Sqrt.
            warm = big.tile([1, 1], F32)
            nc.scalar.activation(warm[:], eps_b[0:1, :], AF.Sqrt)
            loop_cm = tc.For_i(0, reps) if reps > 1 else contextlib.nullcontext()
            with loop_cm:
                _body(nc, tc, scratch, small, psum, v_sb, a_sb, c_sb, eps_b,
                      v_re, a_re, dma_mode, pool_casts, act_relu)
            lscaled = _EPILOGUE_OUT[0]

            if collective:
                loss_bounce = dram.tile([1, 1], F32)
                out_bounce = dram.tile([1, 1], F32)
                nc.sync.dma_start(loss_bounce[:], lscaled[:])
                nc.gpsimd.collective_compute(
                    "AllReduce",
                    OP.add,
                    replica_groups=[list(range(N_CORES))],
                    ins=[loss_bounce.opt()],
                    outs=[out_bounce.opt()],
                )
                nc.sync.dma_start(out_ext[:], out_bounce[:])
            else:
                nc.sync.dma_start(out_ext[:], lscaled[:])

    nc.compile()
    return nc


_EPILOGUE_OUT = [None]


def _body(nc, tc, scratch, small, psum, v_sb, a_sb, c_sb, eps_b,
          v_re, a_re, dma_mode, pool_casts=4, act_relu=False):
    cast = dma_mode == "hwdge_f32"
    # All input DMAs on the single SP HWDGE queue (fastest measured), v
    # then a per chunk so the v tiles land first and their casts/squares
    # overlap the a drain.
    col = 0
    for tpc in _PLAN[0]:
        sl = slice(col * D, (col + tpc) * D)
        if cast:
            nc.sync.dma_start(v_sb[:, sl], v_re[:, sl])
            nc.sync.dma_start(a_sb[:, sl], a_re[:, sl])
        else:
            nc.gpsimd.dma_start(v_sb[:, sl], v_re[:, sl])
            nc.gpsimd.dma_start(a_sb[:, sl], a_re[:, sl])
        col += tpc

    sumv_ps = psum.tile([1, D], F32)
    suma_ps = psum.tile([1, D], F32)
    sync_ps = psum.tile([1, D], F32)

    t0 = 0
    cplan = _CPLAN[0] if _CPLAN[0] is not None else _PLAN[0]
    for ci, tpc in enumerate(cplan):
        first = ci == 0
        last = ci == len(cplan) - 1
        # ss: cols [0:tpc] = sum v^2 per tile, [tpc:2*tpc] = sum a^2
        ss = small.tile([P, 2 * tpc], F32, tag=f"ss{tpc}")
        if _SQ_MODE[0] == "bn":
            stats = small.tile([P, 12 * tpc], F32, tag=f"bns{tpc}")
        vbs, abs_, prods = [], [], []
        # cast engine per (j, tensor): first `pool_casts` of the chunk's
        # 2*tpc casts go to the otherwise-idle Pool engine, rest to DVE
        n_pool = min(pool_casts, 2 * tpc)
        ci_cast = 0
        for j in range(tpc):
            t = t0 + j
            sl = slice(t * D, (t + 1) * D)
            if cast:
                vb_t = scratch.tile([P, D], BF16, tag=f"vb{j}")
                eng = nc.gpsimd if ci_cast < n_pool else nc.vector
                eng.tensor_copy(vb_t[:], v_sb[:, sl])
                ci_cast += 1
                vb = vb_t[:]
                ab_t = scratch.tile([P, D], BF16, tag=f"ab{j}")
                eng = nc.gpsimd if ci_cast < n_pool else nc.vector
                eng.tensor_copy(ab_t[:], a_sb[:, sl])
                ci_cast += 1
                ab = ab_t[:]
            else:
                vb = v_sb[:, sl]
                ab = a_sb[:, sl]
            vbs.append(vb)
            abs_.append(ab)
            if _SQ_MODE[0] == "bn":
                # one DVE pass per tile gives even/odd count/mean/var;
                # sum-of-squares is reconstructed in batched ops below
                nc.vector.bn_stats(stats[:, 6 * j:6 * j + 6], vb)
                nc.vector.bn_stats(
                    stats[:, 6 * (tpc + j):6 * (tpc + j) + 6], ab)
            else:
                # square outputs are junk (only the accums are used); one
                # shared tile per engine — writes are engine-serial anyway.
                # (Tried junk->PSUM: contends with PE accumulation, +9us.)
                # v-squares on ScalarE; a-squares: first `act_asq` per
                # chunk on ScalarE, rest on DVE (scalar_tensor_tensor
                # accum; InstTensorTensorReduce faults on this HW)
                if j < _ACT_VSQ[0]:
                    sq_v = scratch.tile([P, D], BF16, tag="sqj_act")
                    nc.scalar.activation(
                        sq_v[:], vb, AF.Square, accum_out=ss[:, j:j + 1],
                    )
                else:
                    sq_v = scratch.tile([P, D], BF16, tag="sqj_dve")
                    nc.vector.scalar_tensor_tensor(
                        out=sq_v[:], in0=vb, scalar=1.0, in1=vb,
                        op0=OP.mult, op1=OP.mult,
                        accum_out=ss[:, j:j + 1],
                    )
                if j < _ACT_ASQ[0]:
                    nc.scalar.activation(
                        sq_v[:], ab, AF.Square,
                        accum_out=ss[:, tpc + j:tpc + j + 1],
                    )
                else:
                    sq_a = scratch.tile([P, D], BF16, tag="sqj_dve")
                    nc.vector.scalar_tensor_tensor(
                        out=sq_a[:], in0=ab, scalar=1.0, in1=ab,
                        op0=OP.mult, op1=OP.mult,
                        accum_out=ss[:, tpc + j:tpc + j + 1],
                    )



        # prod = v*a (bf16 2x mode); paired 2-tile ops halve the DVE
        # instruction count; its weighted row-sum goes through the PE
        # below, so no per-row dot accum is needed
        jp = 0
        while jp < tpc:
            w = min(_PROD_W[0], tpc - jp) if not cast else 1
            sl2 = slice((t0 + jp) * D, (t0 + jp + w) * D)
            prod = scratch.tile([P, w * D], BF16, tag=f"prod{jp}_{w}")
            nc.vector.tensor_tensor(
                out=prod[:], in0=v_sb[:, sl2] if not cast else vbs[jp],
                in1=a_sb[:, sl2] if not cast else abs_[jp], op=OP.mult,
            )
            for k in range(w):
                prods.append(prod[:, k * D:(k + 1) * D])
            jp += w

        if _SQ_MODE[0] == "bn":
            # ss = cv_e + cv_o + 256*(m_e^2 + m_o^2), batched per chunk
            m_e = stats[:, 1::6]
            m_o = stats[:, 4::6]
            cv_e = stats[:, 2::6]
            cv_o = stats[:, 5::6]
            t1 = small.tile([P, 2 * tpc], F32, tag=f"bt1{tpc}")
            nc.vector.tensor_mul(t1[:], m_e, m_e)
            t2 = small.tile([P, 2 * tpc], F32, tag=f"bt2{tpc}")
            nc.vector.scalar_tensor_tensor(
                out=t2[:], in0=m_o, scalar=1.0, in1=m_o,
                op0=OP.mult, op1=OP.mult,
            )
            t3 = small.tile([P, 2 * tpc], F32, tag=f"bt3{tpc}")
            nc.vector.tensor_add(t3[:], cv_e, cv_o)
            t4 = small.tile([P, 2 * tpc], F32, tag=f"bt4{tpc}")
            nc.vector.tensor_add(t4[:], t1[:], t2[:])
            nc.vector.scalar_tensor_tensor(
                out=ss[:], in0=t4[:], scalar=float(D // 2), in1=t3[:],
                op0=OP.mult, op1=OP.add,
            )

        # Batched 1/max(norm, eps) for the whole chunk. The sqrt bias
        # keeps sqrt(0) finite, matching F.normalize's max(norm, 1e-12)
        # for all realizable inputs.
        nrm = small.tile([P, 2 * tpc], F32, tag=f"nrm{tpc}")
        nc.scalar.activation(nrm[:], ss[:], AF.Sqrt, bias=eps_b[:])
        inv_b = small.tile([P, 2 * tpc], BF16, tag=f"invb{tpc}")
        if _RECIP_BF[0]:
            # bf16 reciprocal directly: the weights feed bf16 matmuls
            # anyway, so the f32 intermediate + copy is pure overhead
            with nc.allow_low_precision("weights are bf16 matmul inputs"):
                nc.vector.reciprocal(inv_b[:], nrm[:])
            invva_b = small.tile([P, tpc], BF16, tag=f"invva{tpc}")
            nc.vector.tensor_mul(invva_b[:], inv_b[:, 0:tpc], inv_b[:, tpc:])
        else:
            inv = small.tile([P, 2 * tpc], F32, tag=f"inv{tpc}")
            nc.vector.reciprocal(inv[:], nrm[:])
            nc.vector.tensor_copy(inv_b[:], inv[:])
            invva_b = small.tile([P, tpc], BF16, tag=f"invva{tpc}")
            nc.vector.tensor_mul(invva_b[:], inv[:, 0:tpc], inv[:, tpc:])

        for j in range(tpc):
            st = first and j == 0
            sp = last and j == tpc - 1
            if sp:
                # final tile: stop suma FIRST so the epilogue's ACT
                # PSUM->SBUF copy of suma overlaps the remaining matmuls
                nc.tensor.matmul(
                    suma_ps[:], lhsT=inv_b[:, tpc + j:tpc + j + 1],
                    rhs=abs_[j], start=st, stop=sp,
                )
                nc.tensor.matmul(
                    sync_ps[:], lhsT=invva_b[:, j:j + 1], rhs=prods[j],
                    start=st, stop=sp,
                )
                nc.tensor.matmul(
                    sumv_ps[:], lhsT=inv_b[:, j:j + 1], rhs=vbs[j],
                    start=st, stop=sp,
                )
            else:
                nc.tensor.matmul(
                    sumv_ps[:], lhsT=inv_b[:, j:j + 1], rhs=vbs[j],
                    start=st, stop=sp,
                )
                nc.tensor.matmul(
                    suma_ps[:], lhsT=inv_b[:, tpc + j:tpc + j + 1],
                    rhs=abs_[j], start=st, stop=sp,
                )
                # sync row: [1,D] += invva.T @ (v*a); summed in epilogue
                nc.tensor.matmul(
                    sync_ps[:], lhsT=invva_b[:, j:j + 1], rhs=prods[j],
                    start=st, stop=sp,
                )
        t0 += tpc

    # ---- epilogue: 5 ops on partition 0 ----
    # t2 = [total, sync]; z = cA*total + cS*sync; out = relu(z + cM)
    suma_sb = small.tile([1, D], F32)
    nc.scalar.copy(suma_sb[:], suma_ps[:])
    t2 = small.tile([1, 2], F32)
    junk1 = scratch.tile([P, D], F32, tag="junk1")
    nc.vector.scalar_tensor_tensor(
        out=junk1[0:1, :], in0=sumv_ps[:], scalar=1.0, in1=suma_sb[:],
        op0=OP.mult, op1=OP.mult, accum_out=t2[:, 0:1],
    )
    nc.vector.tensor_reduce(
        out=t2[:, 1:2], in_=sync_ps[:], op=OP.add,
        axis=mybir.AxisListType.X,
    )
    junk2 = small.tile([1, 2], F32)
    z = small.tile([1, 1], F32)
    nc.vector.scalar_tensor_tensor(
        out=junk2[:], in0=t2[:], scalar=1.0, in1=c_sb[:, 0:2],
        op0=OP.mult, op1=OP.mult, accum_out=z[:],
    )
    lscaled = small.tile([1, 1], F32)
    if act_relu:
        nc.scalar.activation(lscaled[:], z[:], AF.Relu, bias=c_sb[:, 2:3])
    else:
        # DVE relu: avoids a ScalarE activation-table switch on the tail
        zb = small.tile([1, 1], F32)
        nc.vector.tensor_add(zb[:], z[:], c_sb[:, 2:3])
        nc.vector.tensor_scalar_max(lscaled[:], zb[:], 0.0)
    _EPILOGUE_OUT[0] = lscaled


_NC = None


def _get_nc():
    global _NC
    if _NC is None:
        _NC = _build()
    return _NC


def make_in_maps(visual_features, audio_features, targets):
    vf = np.asarray(visual_features)
    af = np.asarray(audio_features)
    tg = np.asarray(targets)
    maps = []
    for i in range(N_CORES):
        w = float(tg[i])
        sgn = 2.0 * w - 1.0
        cA = sgn * C_ASYNC / N_CORES
        cS = -sgn * (C_SYNC + C_ASYNC) / N_CORES
        cM = (0.1 + 0.9 * w) * MARGIN / N_CORES
        maps.append(
            {
                "v": np.ascontiguousarray(vf[i], dtype=np.float32),
                "a": np.ascontiguousarray(af[i], dtype=np.float32),
                "c": np.array([[cA, cS, cM]], dtype=np.float32),
            }
        )
    return maps


def kernel(visual_features, audio_features, targets):
    nc = _get_nc()
    in_maps = make_in_maps(visual_features, audio_features, targets)
    res = run_bass_kernel_spmd(nc, in_maps, core_ids=list(range(N_CORES)))
    out = np.asarray(res.results[0]["out"], dtype=np.float32)
    return out.reshape(())


if __name__ == "__main__":
    rng = np.random.default_rng(0)
    v = rng.standard_normal((N_CORES, S, D)).astype(np.float32)
    a = rng.standard_normal((N_CORES, S, D)).astype(np.float32)
    t = rng.integers(0, 2, (N_CORES,)).astype(np.int32)
    print(kernel(visual_features=v, audio_features=a, targets=t))
